# revision 11
# baseline (speedup 1.0000x reference)
"""Trainium2 Bass kernel for nn_MultiHeadAttention_40286793236532.

Single-head attention with a mixed-precision QKV projection:
  qkv = x @ w_qkv   (contraction split fp16 | fp32 | fp16 over bands
                     [0:256) [256:768) [768:1024))
  q, k, v = split(qkv); s = softmax(q k^T / 32); out = (s v) @ w_out^T + b

Sharding: data-parallel over batch B=8 -> one batch element per NeuronCore,
no collectives. Each core runs the identical program on its x-slice.

Per-core algorithm (N=2048 tokens, d=1024):
  Phase A: transpose x on-chip (PE transpose; low bands cast to fp16 in the
    PSUM->SBUF copy), then QKV matmuls: fp16 matmuls for the low contraction
    bands, float32r (full-rate, ~1.6e-4 rel err) for the high band.
    Q^T, K^T written transposed [d, N]; V written natural [N, d]; all to DRAM
    scratch. W_out is PE-transposed to DRAM as well.
  Phase B: K^T and V resident in SBUF; per 256-query block:
    S^T tiles = K-tile^T . Q-block (f32r, keys on partitions), exp on ACT with
    scale=1/32 folded in (no max subtraction: |logits| <~ 7, safe in fp32),
    Y^T accumulated over key tiles in PSUM (lhsT = V tile), row sums via a
    ones-column matmul, then out = Y^T.T @ W_out^T with a fused epilogue
    (x * 1/sum + bias) and direct DMA to the output.
"""

import numpy as np

import concourse.bacc as bacc
import concourse.bass as bass
import concourse.mybir as mybir
import concourse.tile as tile
from concourse.bass_utils import run_bass_kernel_spmd
from concourse.masks import make_identity

F32 = mybir.dt.float32
F32R = mybir.dt.float32r
F16 = mybir.dt.float16

B, N, D = 8, 2048, 1024
KL = 256          # low-precision band width (each side)
NT = N // 128     # 16 token tiles
DT = D // 128     # 8 contraction k-tiles
QBLK = 256        # queries per phase-B block
NBLK = N // QBLK  # 8 blocks
# global k-tile index -> (is_low, local index within WH/WL, XTH/XTL)
# low k-tiles: 0,1 (cols 0:256) and 6,7 (cols 768:1024); high: 2..5
KMAP = {0: (True, 0), 1: (True, 1), 2: (False, 0), 3: (False, 1),
        4: (False, 2), 5: (False, 3), 6: (True, 2), 7: (True, 3)}
LOW_ROWS = [0, 128, 768, 896]  # first row of each low k-tile in w_qkv/x cols


def _r(dt_, ap):
    """bitcast an fp32 AP to float32r for full-rate PE matmuls"""
    return ap.bitcast(dt_)


def build_nc():
    nc = bacc.Bacc()
    x_d = nc.dram_tensor("x", [N, D], F32, kind="ExternalInput")
    wqkv_d = nc.dram_tensor("weight_qkv", [D, 3 * D], F32, kind="ExternalInput")
    wout_d = nc.dram_tensor("out_w", [D, D], F32, kind="ExternalInput")
    bout_d = nc.dram_tensor("out_b", [D], F32, kind="ExternalInput")
    out_d = nc.dram_tensor("out", [N, D], F32, kind="ExternalOutput")

    with tile.TileContext(nc) as tc:
        with tc.tile_pool(name="dram", bufs=1, space="DRAM") as dram, \
             tc.tile_pool(name="persist", bufs=1) as persist:
            qT = dram.tile([D, N], F32R)      # Q^T scratch
            kT = dram.tile([D, N], F32R)      # K^T scratch
            vN = dram.tile([N, D], F32R)      # V natural scratch
            wT = dram.tile([D, D], F32R)      # W_out^T scratch

            ident = persist.tile([128, 128], F32)
            make_identity(nc, ident)
            ident1 = persist.tile([1, 1], F32)
            nc.vector.memset(ident1, 1.0)
            ones_f = persist.tile([128, 1], F32)
            nc.vector.memset(ones_f, 1.0)
            ones = persist.tile([128, 1], F32R)
            nc.vector.tensor_copy(out=ones, in_=ones_f)
            bias = persist.tile([128, D], F32)
            bias_bcast = bass.AP(
                tensor=bout_d, offset=0,
                ap=[[0, 128], [1, D]],
            )
            nc.sync.dma_start(out=bias, in_=bias_bcast)

            # ---------------- Phase A ----------------
            with tc.tile_pool(name="pa_big", bufs=1) as pa, \
                 tc.tile_pool(name="pa_stage", bufs=3) as stage, \
                 tc.tile_pool(name="pa_wstage", bufs=2) as wstage, \
                 tc.tile_pool(name="pa_pst", bufs=2, space="PSUM") as pst, \
                 tc.tile_pool(name="pa_psmm", bufs=4, space="PSUM") as psmm:

                WH = pa.tile([128, 4, 3 * D], F32R)   # w rows 256:768
                WL = pa.tile([128, 4, 3 * D], F16)   # w rows 0:256 + 768:1024
                XTH = pa.tile([128, 4, N], F32R)      # x^T high band
                XTL = pa.tile([128, 4, N], F16)      # x^T low bands

                nc.sync.dma_start(
                    out=WH,
                    in_=wqkv_d.ap()[256:768].rearrange("(t p) c -> p t c", p=128).bitcast(F32R))
                for li in range(4):
                    r0 = LOW_ROWS[li]
                    ws = wstage.tile([128, 3 * D], F32, tag="wstage")
                    nc.sync.dma_start(out=ws, in_=wqkv_d.ap()[r0:r0 + 128])
                    nc.any.tensor_copy(out=WL[:, li], in_=ws)  # cast f32->f16

                # x transpose pipeline + QKV matmuls, grouped per 512 tokens
                for f in range(4):
                    for tt in range(4 * f, 4 * f + 4):
                        xn = stage.tile([128, D], F32, tag="xnat")
                        nc.sync.dma_start(out=xn, in_=x_d.ap()[tt * 128:(tt + 1) * 128])
                        tsl = slice(tt * 128, (tt + 1) * 128)
                        for j in range(4):  # high band k-tiles
                            tp = pst.tile([128, 128], F32, tag="tp")
                            nc.tensor.transpose(
                                tp, xn[:, 256 + j * 128: 256 + (j + 1) * 128], ident)
                            nc.any.tensor_copy(out=XTH[:, j, tsl], in_=tp)
                        for li in range(4):  # low band k-tiles (cast in copy)
                            r0 = LOW_ROWS[li]
                            tp = pst.tile([128, 128], F32, tag="tp")
                            nc.tensor.transpose(tp, xn[:, r0:r0 + 128], ident)
                            nc.any.tensor_copy(out=XTL[:, li, tsl], in_=tp)

                    tok = slice(f * 512, (f + 1) * 512)
                    # Q^T and K^T: lhsT = w tile (stationary), rhs = x^T
                    for m in range(16):  # 8 Q dh-tiles then 8 K dh-tiles
                        csl = slice(m * 128, (m + 1) * 128)
                        ps = psmm.tile([128, 512], F32, tag="mm")
                        for kt in range(DT):
                            low, li = KMAP[kt]
                            if low:
                                nc.tensor.matmul(
                                    ps, WL[:, li, csl], XTL[:, li, tok],
                                    start=(kt == 0), stop=(kt == DT - 1))
                            else:
                                nc.tensor.matmul(
                                    ps, WH[:, li, csl],
                                    XTH[:, li, tok],
                                    start=(kt == 0), stop=(kt == DT - 1))
                        dst = qT if m < 8 else kT
                        rsl = slice((m % 8) * 128, (m % 8 + 1) * 128)
                        st = stage.tile([128, 512], F32R, tag="qkvout")
                        nc.any.tensor_copy(out=st, in_=ps)
                        nc.sync.dma_start(out=dst[rsl, tok], in_=st)
                    # V natural: lhsT = x^T tile (stationary), rhs = w_v
                    for t in range(4 * f, 4 * f + 4):
                        tsl = slice(t * 128, (t + 1) * 128)
                        for h in range(2):
                            vsl = slice(2 * D + h * 512, 2 * D + (h + 1) * 512)
                            ps = psmm.tile([128, 512], F32, tag="mm")
                            for kt in range(DT):
                                low, li = KMAP[kt]
                                if low:
                                    nc.tensor.matmul(
                                        ps, XTL[:, li, tsl], WL[:, li, vsl],
                                        start=(kt == 0), stop=(kt == DT - 1))
                                else:
                                    nc.tensor.matmul(
                                        ps, XTH[:, li, tsl],
                                        WH[:, li, vsl],
                                        start=(kt == 0), stop=(kt == DT - 1))
                            st = stage.tile([128, 512], F32R, tag="qkvout")
                            nc.any.tensor_copy(out=st, in_=ps)
                            nc.sync.dma_start(
                                out=vN[t * 128:(t + 1) * 128, h * 512:(h + 1) * 512],
                                in_=st)

                # W_out^T via PE transpose -> DRAM scratch
                for et in range(8):
                    wn = stage.tile([128, D], F32, tag="woutnat")
                    nc.sync.dma_start(out=wn, in_=wout_d.ap()[et * 128:(et + 1) * 128])
                    for j in range(8):
                        tp = pst.tile([128, 128], F32, tag="tp")
                        nc.tensor.transpose(tp, wn[:, j * 128:(j + 1) * 128], ident)
                        st = stage.tile([128, 128], F32R, tag="woutT")
                        nc.any.tensor_copy(out=st, in_=tp)
                        nc.sync.dma_start(
                            out=wT[j * 128:(j + 1) * 128, et * 128:(et + 1) * 128],
                            in_=st)

            # ---------------- Phase B ----------------
            with tc.tile_pool(name="pb_big", bufs=1) as pb, \
                 tc.tile_pool(name="pb_q", bufs=2) as pq, \
                 tc.tile_pool(name="pb_w", bufs=2) as pw, \
                 tc.tile_pool(name="pb_p", bufs=3) as ppt, \
                 tc.tile_pool(name="pb_y", bufs=1) as py, \
                 tc.tile_pool(name="pb_o", bufs=3) as po, \
                 tc.tile_pool(name="pb_misc", bufs=2) as pmisc, \
                 tc.tile_pool(name="pb_psy", bufs=1, space="PSUM") as psy, \
                 tc.tile_pool(name="pb_pssum", bufs=1, space="PSUM") as pssum, \
                 tc.tile_pool(name="pb_pss", bufs=3, space="PSUM") as pss:

                Ksb = pb.tile([128, DT, N], F32R)
                nc.sync.dma_start(
                    out=Ksb, in_=kT[:].rearrange("(t p) n -> p t n", p=128))
                Vsb = pb.tile([128, NT, D], F32R)
                nc.sync.dma_start(
                    out=Vsb, in_=vN[:].rearrange("(t p) d -> p t d", p=128))

                for b in range(NBLK):
                    q0 = b * QBLK
                    Qb = pq.tile([128, DT, QBLK], F32R, tag="qb")
                    nc.sync.dma_start(
                        out=Qb,
                        in_=qT[:, q0:q0 + QBLK].rearrange("(t p) q -> p t q", p=128))

                    yt_ps = psy.tile([128, DT, QBLK], F32, tag="yt")
                    # zero the accumulator and use start=False on every matmul:
                    # correct for any PE ordering / stale has_written state
                    # (accumulate-onto-0 and overwrite-with-product agree)
                    nc.vector.memset(yt_ps, 0.0)
                    sums_ps = pssum.tile([1, QBLK], F32, tag="sums")

                    for j in range(NT):  # key tiles
                        ksl = slice(j * 128, (j + 1) * 128)
                        s_ps = pss.tile([128, QBLK], F32, tag="small")
                        for kt in range(DT):
                            nc.tensor.matmul(
                                s_ps, Ksb[:, kt, ksl],
                                Qb[:, kt],
                                start=(kt == 0), stop=(kt == DT - 1))
                        pt = ppt.tile([128, QBLK], F32R, tag="pt")
                        nc.scalar.activation(
                            out=pt, in_=s_ps,
                            func=mybir.ActivationFunctionType.Exp,
                            scale=1.0 / 32.0)
                        for m in range(DT):
                            # one start per 2KB PSUM zero-region (= 2 m-slices)
                            nc.tensor.matmul(
                                yt_ps[:, m],
                                Vsb[:, j, m * 128:(m + 1) * 128],
                                pt,
                                start=False, stop=(j == NT - 1),
                                skip_group_check=True)
                        nc.tensor.matmul(
                            sums_ps, ones, pt,
                            start=(j == 0), stop=(j == NT - 1),
                            skip_group_check=True)

                    # 1/rowsum as a per-partition column, via PE transpose
                    sums_sb = pmisc.tile([1, QBLK], F32, tag="sums_sb")
                    nc.any.tensor_copy(out=sums_sb, in_=sums_ps)
                    recip = pmisc.tile([128, 2], F32, tag="recip")
                    for t in range(2):
                        rp = pss.tile([128, 1], F32, tag="small")
                        nc.tensor.transpose(
                            rp, sums_sb[0:1, t * 128:(t + 1) * 128], ident1)
                        nc.vector.reciprocal(out=recip[:, t:t + 1], in_=rp)

                    yt_sb = py.tile([128, DT, QBLK], F32R, tag="yt_sb")
                    nc.any.tensor_copy(out=yt_sb, in_=yt_ps)

                    # out projection + fused epilogue
                    for e4 in range(4):
                        esl = slice(e4 * 256, (e4 + 1) * 256)
                        wTe = pw.tile([128, DT, 256], F32R, tag="wte")
                        nc.sync.dma_start(
                            out=wTe,
                            in_=wT[:, esl].rearrange("(t p) e -> p t e", p=128))
                        for t in range(2):
                            qsl = slice(t * 128, (t + 1) * 128)
                            o_ps = pss.tile([128, QBLK], F32, tag="small")
                            for kt in range(DT):
                                nc.tensor.matmul(
                                    o_ps, yt_sb[:, kt, qsl],
                                    wTe[:, kt],
                                    start=(kt == 0), stop=(kt == DT - 1))
                            o_sb = po.tile([128, 256], F32, tag="osb")
                            nc.vector.scalar_tensor_tensor(
                                out=o_sb, in0=o_ps, scalar=recip[:, t:t + 1],
                                in1=bias[:, esl],
                                op0=mybir.AluOpType.mult,
                                op1=mybir.AluOpType.add)
                            nc.sync.dma_start(
                                out=out_d.ap()[q0 + t * 128: q0 + (t + 1) * 128, esl],
                                in_=o_sb)
    nc.finalize()
    return nc


_NC = None


def kernel(**inputs) -> np.ndarray:
    global _NC
    if _NC is None:
        _NC = build_nc()
    x = np.ascontiguousarray(inputs["x"], dtype=np.float32)
    w = np.ascontiguousarray(inputs["weight_qkv"], dtype=np.float32)
    ow = np.ascontiguousarray(inputs["out_w"], dtype=np.float32)
    ob = np.ascontiguousarray(inputs["out_b"], dtype=np.float32)
    in_maps = [
        {"x": x[i], "weight_qkv": w, "out_w": ow, "out_b": ob} for i in range(B)
    ]
    res = run_bass_kernel_spmd(_NC, in_maps, core_ids=list(range(B)))
    return np.stack([res.results[i]["out"] for i in range(B)], axis=0)


if __name__ == "__main__":
    rng = np.random.default_rng(0)
    ins = {
        "x": rng.standard_normal((B, N, D), dtype=np.float32),
        "weight_qkv": (rng.standard_normal((D, 3 * D)) * D ** -0.5).astype(np.float32),
        "out_w": (rng.standard_normal((D, D)) * D ** -0.5).astype(np.float32),
        "out_b": (rng.standard_normal(D) * 0.01).astype(np.float32),
    }
    out = kernel(**ins)
    print(out.shape, out.dtype)


# revision 17
# speedup vs baseline: 10064.7324x; 10064.7324x over previous
"""Trainium2 Bass kernel for nn_MultiHeadAttention_40286793236532.

Single-head attention with a mixed-precision QKV projection:
  qkv = x @ w_qkv   (contraction split fp16 | fp32 | fp16 over bands
                     [0:256) [256:768) [768:1024))
  q, k, v = split(qkv); s = softmax(q k^T / 32); out = (s v) @ w_out^T + b

Sharding: data-parallel over batch B=8 -> one batch element per NeuronCore,
no collectives. Each core runs the identical program on its x-slice.

Per-core algorithm (N=2048 tokens, d=1024):
  Phase A: transpose x on-chip (PE transpose; low bands cast to fp16 in the
    PSUM->SBUF copy), then QKV matmuls: fp16 matmuls for the low contraction
    bands, float32r (full-rate, ~1.6e-4 rel err) for the high band.
    Q^T, K^T written transposed [d, N]; V written natural [N, d]; all to DRAM
    scratch. W_out is PE-transposed to DRAM as well.
  Phase B: K^T and V resident in SBUF; per 256-query block:
    S^T tiles = K-tile^T . Q-block (f32r, keys on partitions), exp on ACT with
    scale=1/32 folded in (no max subtraction: |logits| <~ 7, safe in fp32),
    Y^T accumulated over key tiles in PSUM (lhsT = V tile), row sums via a
    ones-column matmul, then out = Y^T.T @ W_out^T with a fused epilogue
    (x * 1/sum + bias) and direct DMA to the output.
"""

import numpy as np

import concourse.bacc as bacc
import concourse.bass as bass
import concourse.mybir as mybir
import concourse.tile as tile
from concourse.bass_utils import run_bass_kernel_spmd
from concourse.masks import make_identity

F32 = mybir.dt.float32
F32R = mybir.dt.float32r
F16 = mybir.dt.float16

B, N, D = 8, 2048, 1024
KL = 256          # low-precision band width (each side)
NT = N // 128     # 16 token tiles
DT = D // 128     # 8 contraction k-tiles
QBLK = 256        # queries per phase-B block
NBLK = N // QBLK  # 8 blocks
# global k-tile index -> (is_low, local index within WH/WL, XTH/XTL)
# low k-tiles: 0,1 (cols 0:256) and 6,7 (cols 768:1024); high: 2..5
KMAP = {0: (True, 0), 1: (True, 1), 2: (False, 0), 3: (False, 1),
        4: (False, 2), 5: (False, 3), 6: (True, 2), 7: (True, 3)}
LOW_ROWS = [0, 128, 768, 896]  # first row of each low k-tile in w_qkv/x cols


def _r(dt_, ap):
    """bitcast an fp32 AP to float32r for full-rate PE matmuls"""
    return ap.bitcast(dt_)


def build_nc():
    nc = bacc.Bacc()
    x_d = nc.dram_tensor("x", [N, D], F32, kind="ExternalInput")
    wqkv_d = nc.dram_tensor("weight_qkv", [D, 3 * D], F32, kind="ExternalInput")
    wout_d = nc.dram_tensor("out_w", [D, D], F32, kind="ExternalInput")
    bout_d = nc.dram_tensor("out_b", [D], F32, kind="ExternalInput")
    out_d = nc.dram_tensor("out", [N, D], F32, kind="ExternalOutput")

    with tile.TileContext(nc) as tc:
        with tc.tile_pool(name="dram", bufs=1, space="DRAM") as dram, \
             tc.tile_pool(name="persist", bufs=1) as persist:
            qT = dram.tile([D, N], F32R)      # Q^T scratch
            kT = dram.tile([D, N], F32R)      # K^T scratch
            vN = dram.tile([N, D], F32R)      # V natural scratch
            wT = dram.tile([D, D], F32R)      # W_out^T scratch

            ident = persist.tile([128, 128], F32)
            make_identity(nc, ident)
            ident1 = persist.tile([1, 1], F32)
            nc.vector.memset(ident1, 1.0)
            ones_f = persist.tile([128, 1], F32)
            nc.vector.memset(ones_f, 1.0)
            ones = persist.tile([128, 1], F32R)
            nc.vector.tensor_copy(out=ones, in_=ones_f)
            bias = persist.tile([128, D], F32)
            bias_bcast = bass.AP(
                tensor=bout_d, offset=0,
                ap=[[0, 128], [1, D]],
            )
            nc.sync.dma_start(out=bias, in_=bias_bcast)

            # ---------------- Phase A ----------------
            with tc.tile_pool(name="pa_big", bufs=1) as pa, \
                 tc.tile_pool(name="pa_stage", bufs=3) as stage, \
                 tc.tile_pool(name="pa_wstage", bufs=2) as wstage, \
                 tc.tile_pool(name="pa_pst", bufs=2, space="PSUM") as pst, \
                 tc.tile_pool(name="pa_psmm", bufs=4, space="PSUM") as psmm:

                WH = pa.tile([128, 4, 3 * D], F32R)   # w rows 256:768
                WL = pa.tile([128, 4, 3 * D], F16)   # w rows 0:256 + 768:1024
                XTH = pa.tile([128, 4, N], F32R)      # x^T high band
                XTL = pa.tile([128, 4, N], F16)      # x^T low bands

                def emit_xpipe(f):
                    # x tiles DMA'd, PE-transposed; low bands cast in the copy
                    for tt in range(4 * f, 4 * f + 4):
                        xn = stage.tile([128, D], F32, tag="xnat")
                        nc.sync.dma_start(out=xn, in_=x_d.ap()[tt * 128:(tt + 1) * 128])
                        tsl = slice(tt * 128, (tt + 1) * 128)
                        for j in range(4):  # high band k-tiles
                            tp = pst.tile([128, 128], F32, tag="tp")
                            nc.tensor.transpose(
                                tp, xn[:, 256 + j * 128: 256 + (j + 1) * 128], ident)
                            nc.any.tensor_copy(out=XTH[:, j, tsl], in_=tp)
                        for li in range(4):  # low band k-tiles (cast in copy)
                            r0 = LOW_ROWS[li]
                            tp = pst.tile([128, 128], F32, tag="tp")
                            nc.tensor.transpose(tp, xn[:, r0:r0 + 128], ident)
                            nc.any.tensor_copy(out=XTL[:, li, tsl], in_=tp)

                # x DMAs + transposes first (independent of W); W loads follow
                emit_xpipe(0)
                nc.sync.dma_start(
                    out=WH,
                    in_=wqkv_d.ap()[256:768].rearrange("(t p) c -> p t c", p=128).bitcast(F32R))
                for li in range(4):
                    r0 = LOW_ROWS[li]
                    ws = wstage.tile([128, 3 * D], F32, tag="wstage")
                    nc.sync.dma_start(out=ws, in_=wqkv_d.ap()[r0:r0 + 128])
                    nc.any.tensor_copy(out=WL[:, li], in_=ws)  # cast f32->f16

                # QKV matmuls per 512-token group, x-pipe one group ahead
                for f in range(4):
                    if f + 1 < 4:
                        emit_xpipe(f + 1)
                    tok = slice(f * 512, (f + 1) * 512)
                    # Q^T and K^T: lhsT = w tile (stationary), rhs = x^T
                    for m in range(16):  # 8 Q dh-tiles then 8 K dh-tiles
                        csl = slice(m * 128, (m + 1) * 128)
                        ps = psmm.tile([128, 512], F32, tag="mm")
                        for kt in range(DT):
                            low, li = KMAP[kt]
                            if low:
                                nc.tensor.matmul(
                                    ps, WL[:, li, csl], XTL[:, li, tok],
                                    start=(kt == 0), stop=(kt == DT - 1))
                            else:
                                nc.tensor.matmul(
                                    ps, WH[:, li, csl],
                                    XTH[:, li, tok],
                                    start=(kt == 0), stop=(kt == DT - 1))
                        dst = qT if m < 8 else kT
                        rsl = slice((m % 8) * 128, (m % 8 + 1) * 128)
                        st = stage.tile([128, 512], F32R, tag="qkvout")
                        nc.any.tensor_copy(out=st, in_=ps)
                        nc.sync.dma_start(out=dst[rsl, tok], in_=st)
                    # V natural: lhsT = x^T tile (stationary), rhs = w_v
                    for t in range(4 * f, 4 * f + 4):
                        tsl = slice(t * 128, (t + 1) * 128)
                        for h in range(2):
                            vsl = slice(2 * D + h * 512, 2 * D + (h + 1) * 512)
                            ps = psmm.tile([128, 512], F32, tag="mm")
                            for kt in range(DT):
                                low, li = KMAP[kt]
                                if low:
                                    nc.tensor.matmul(
                                        ps, XTL[:, li, tsl], WL[:, li, vsl],
                                        start=(kt == 0), stop=(kt == DT - 1))
                                else:
                                    nc.tensor.matmul(
                                        ps, XTH[:, li, tsl],
                                        WH[:, li, vsl],
                                        start=(kt == 0), stop=(kt == DT - 1))
                            st = stage.tile([128, 512], F32R, tag="qkvout")
                            nc.any.tensor_copy(out=st, in_=ps)
                            nc.sync.dma_start(
                                out=vN[t * 128:(t + 1) * 128, h * 512:(h + 1) * 512],
                                in_=st)

                # W_out^T via PE transpose -> DRAM scratch
                for et in range(8):
                    wn = stage.tile([128, D], F32, tag="woutnat")
                    nc.sync.dma_start(out=wn, in_=wout_d.ap()[et * 128:(et + 1) * 128])
                    for j in range(8):
                        tp = pst.tile([128, 128], F32, tag="tp")
                        nc.tensor.transpose(tp, wn[:, j * 128:(j + 1) * 128], ident)
                        st = stage.tile([128, 128], F32R, tag="woutT")
                        nc.any.tensor_copy(out=st, in_=tp)
                        nc.sync.dma_start(
                            out=wT[j * 128:(j + 1) * 128, et * 128:(et + 1) * 128],
                            in_=st)

            # ---------------- Phase B ----------------
            with tc.tile_pool(name="pb_big", bufs=1) as pb, \
                 tc.tile_pool(name="pb_q", bufs=2) as pq, \
                 tc.tile_pool(name="pb_w", bufs=2) as pw, \
                 tc.tile_pool(name="pb_p", bufs=3) as ppt, \
                 tc.tile_pool(name="pb_y", bufs=1) as py, \
                 tc.tile_pool(name="pb_o", bufs=3) as po, \
                 tc.tile_pool(name="pb_misc", bufs=2) as pmisc, \
                 tc.tile_pool(name="pb_psy", bufs=1, space="PSUM") as psy, \
                 tc.tile_pool(name="pb_pssum", bufs=1, space="PSUM") as pssum, \
                 tc.tile_pool(name="pb_pss", bufs=3, space="PSUM") as pss:

                # chunk K/V loads by key-tile so block 0's S/Y matmuls can
                # start as soon as the first chunks land (the monolithic load
                # serialized ~56us of PE idle at the phase boundary)
                Ksb = pb.tile([128, DT, N], F32R)
                for c in range(8):
                    ksl = slice(c * 256, (c + 1) * 256)
                    nc.sync.dma_start(
                        out=Ksb[:, :, ksl],
                        in_=kT[:, ksl].rearrange("(t p) n -> p t n", p=128))
                Vsb = pb.tile([128, NT, D], F32R)
                for c in range(8):
                    nc.sync.dma_start(
                        out=Vsb[:, 2 * c:2 * c + 2],
                        in_=vN[c * 256:(c + 1) * 256].rearrange(
                            "(t p) d -> p t d", p=128))

                for b in range(NBLK):
                    q0 = b * QBLK
                    Qb = pq.tile([128, DT, QBLK], F32R, tag="qb")
                    nc.sync.dma_start(
                        out=Qb,
                        in_=qT[:, q0:q0 + QBLK].rearrange("(t p) q -> p t q", p=128))

                    yt_ps = psy.tile([128, DT, QBLK], F32, tag="yt")
                    # zero the accumulator and use start=False on every matmul:
                    # correct for any PE ordering / stale has_written state
                    # (accumulate-onto-0 and overwrite-with-product agree)
                    nc.vector.memset(yt_ps, 0.0)
                    sums_ps = pssum.tile([1, QBLK], F32, tag="sums")

                    def emit_s(j):
                        ksl = slice(j * 128, (j + 1) * 128)
                        s_ps = pss.tile([128, QBLK], F32, tag="small")
                        for kt in range(DT):
                            nc.tensor.matmul(
                                s_ps, Ksb[:, kt, ksl],
                                Qb[:, kt],
                                start=(kt == 0), stop=(kt == DT - 1))
                        pt = ppt.tile([128, QBLK], F32R, tag="pt")
                        nc.scalar.activation(
                            out=pt, in_=s_ps,
                            func=mybir.ActivationFunctionType.Exp,
                            scale=1.0 / 32.0)
                        return pt

                    def emit_y(j, pt):
                        for m in range(DT):
                            nc.tensor.matmul(
                                yt_ps[:, m],
                                Vsb[:, j, m * 128:(m + 1) * 128],
                                pt,
                                start=False, stop=(j == NT - 1),
                                skip_group_check=True)
                        nc.tensor.matmul(
                            sums_ps, ones, pt,
                            start=(j == 0), stop=(j == NT - 1),
                            skip_group_check=True)

                    # software pipeline: PE computes S(j+1) while ACT exps j
                    pt_prev = emit_s(0)
                    for j in range(1, NT):
                        pt_j = emit_s(j)
                        emit_y(j - 1, pt_prev)
                        pt_prev = pt_j
                    emit_y(NT - 1, pt_prev)

                    # 1/rowsum as a per-partition column, via PE transpose
                    sums_sb = pmisc.tile([1, QBLK], F32, tag="sums_sb")
                    nc.any.tensor_copy(out=sums_sb, in_=sums_ps)
                    recip = pmisc.tile([128, 2], F32, tag="recip")
                    for t in range(2):
                        rp = pss.tile([128, 1], F32, tag="small")
                        nc.tensor.transpose(
                            rp, sums_sb[0:1, t * 128:(t + 1) * 128], ident1)
                        nc.vector.reciprocal(out=recip[:, t:t + 1], in_=rp)

                    yt_sb = py.tile([128, DT, QBLK], F32R, tag="yt_sb")
                    for m in range(DT):
                        nc.any.tensor_copy(out=yt_sb[:, m], in_=yt_ps[:, m])

                    # out projection + fused epilogue
                    for e4 in range(4):
                        esl = slice(e4 * 256, (e4 + 1) * 256)
                        wTe = pw.tile([128, DT, 256], F32R, tag="wte")
                        nc.sync.dma_start(
                            out=wTe,
                            in_=wT[:, esl].rearrange("(t p) e -> p t e", p=128))
                        for t in range(2):
                            qsl = slice(t * 128, (t + 1) * 128)
                            o_ps = pss.tile([128, QBLK], F32, tag="small")
                            for kt in range(DT):
                                nc.tensor.matmul(
                                    o_ps, yt_sb[:, kt, qsl],
                                    wTe[:, kt],
                                    start=(kt == 0), stop=(kt == DT - 1))
                            o_sb = po.tile([128, 256], F32, tag="osb")
                            nc.vector.scalar_tensor_tensor(
                                out=o_sb, in0=o_ps, scalar=recip[:, t:t + 1],
                                in1=bias[:, esl],
                                op0=mybir.AluOpType.mult,
                                op1=mybir.AluOpType.add)
                            nc.sync.dma_start(
                                out=out_d.ap()[q0 + t * 128: q0 + (t + 1) * 128, esl],
                                in_=o_sb)
    nc.finalize()
    return nc


_NC = None


def kernel(**inputs) -> np.ndarray:
    global _NC
    if _NC is None:
        _NC = build_nc()
    x = np.ascontiguousarray(inputs["x"], dtype=np.float32)
    w = np.ascontiguousarray(inputs["weight_qkv"], dtype=np.float32)
    ow = np.ascontiguousarray(inputs["out_w"], dtype=np.float32)
    ob = np.ascontiguousarray(inputs["out_b"], dtype=np.float32)
    in_maps = [
        {"x": x[i], "weight_qkv": w, "out_w": ow, "out_b": ob} for i in range(B)
    ]
    res = run_bass_kernel_spmd(_NC, in_maps, core_ids=list(range(B)))
    return np.stack([res.results[i]["out"] for i in range(B)], axis=0)


if __name__ == "__main__":
    rng = np.random.default_rng(0)
    ins = {
        "x": rng.standard_normal((B, N, D), dtype=np.float32),
        "weight_qkv": (rng.standard_normal((D, 3 * D)) * D ** -0.5).astype(np.float32),
        "out_w": (rng.standard_normal((D, D)) * D ** -0.5).astype(np.float32),
        "out_b": (rng.standard_normal(D) * 0.01).astype(np.float32),
    }
    out = kernel(**ins)
    print(out.shape, out.dtype)


# revision 23
# speedup vs baseline: 12216.7127x; 1.2138x over previous
"""Trainium2 Bass kernel for nn_MultiHeadAttention_40286793236532.

Single-head attention with a mixed-precision QKV projection:
  qkv = x @ w_qkv   (contraction split fp16 | fp32 | fp16 over bands
                     [0:256) [256:768) [768:1024))
  q, k, v = split(qkv); s = softmax(q k^T / 32); out = (s v) @ w_out^T + b

Sharding: data-parallel over batch B=8 -> one batch element per NeuronCore,
no collectives. Each core runs the identical program on its x-slice.

Per-core algorithm (N=2048 tokens, d=1024):
  Phase A: transpose x on-chip (PE transpose; low bands cast to fp16 in the
    PSUM->SBUF copy), then QKV matmuls: fp16 matmuls for the low contraction
    bands, float32r (full-rate, ~1.6e-4 rel err) for the high band.
    Q^T, K^T written transposed [d, N]; V written natural [N, d]; all to DRAM
    scratch. W_out is PE-transposed to DRAM as well.
  Phase B: K^T and V resident in SBUF; per 256-query block:
    S^T tiles = K-tile^T . Q-block (f32r, keys on partitions), exp on ACT with
    scale=1/32 folded in (no max subtraction: |logits| <~ 7, safe in fp32),
    Y^T accumulated over key tiles in PSUM (lhsT = V tile), row sums via a
    ones-column matmul, then out = Y^T.T @ W_out^T with a fused epilogue
    (x * 1/sum + bias) and direct DMA to the output.
"""

import numpy as np

import concourse.bacc as bacc
import concourse.bass as bass
import concourse.mybir as mybir
import concourse.tile as tile
from concourse.bass_utils import run_bass_kernel_spmd
from concourse.masks import make_identity

F32 = mybir.dt.float32
F32R = mybir.dt.float32r
F16 = mybir.dt.float16

B, N, D = 8, 2048, 1024
KL = 256          # low-precision band width (each side)
NT = N // 128     # 16 token tiles
DT = D // 128     # 8 contraction k-tiles
QBLK = 256        # queries per phase-B block
NBLK = N // QBLK  # 8 blocks
# global k-tile index -> (is_low, local index within WH/WL, XTH/XTL)
# low k-tiles: 0,1 (cols 0:256) and 6,7 (cols 768:1024); high: 2..5
KMAP = {0: (True, 0), 1: (True, 1), 2: (False, 0), 3: (False, 1),
        4: (False, 2), 5: (False, 3), 6: (True, 2), 7: (True, 3)}
KORDER = [0, 1, 6, 7, 2, 3, 4, 5]  # fp16 tiles first: mms start before WH lands
LOW_ROWS = [0, 128, 768, 896]  # first row of each low k-tile in w_qkv/x cols


def _r(dt_, ap):
    """bitcast an fp32 AP to float32r for full-rate PE matmuls"""
    return ap.bitcast(dt_)


def build_nc():
    nc = bacc.Bacc()
    x_d = nc.dram_tensor("x", [N, D], F32, kind="ExternalInput")
    wqkv_d = nc.dram_tensor("weight_qkv", [D, 3 * D], F32, kind="ExternalInput")
    wout_d = nc.dram_tensor("out_w", [D, D], F32, kind="ExternalInput")
    bout_d = nc.dram_tensor("out_b", [D], F32, kind="ExternalInput")
    out_d = nc.dram_tensor("out", [N, D], F32, kind="ExternalOutput")

    with tile.TileContext(nc) as tc:
        with tc.tile_pool(name="dram", bufs=1, space="DRAM") as dram, \
             tc.tile_pool(name="persist", bufs=1) as persist:
            qT = dram.tile([D, N], F32R)      # Q^T scratch
            vN = dram.tile([N, D], F32R)      # V natural scratch

            ident = persist.tile([128, 128], F32)
            make_identity(nc, ident)
            ident1 = persist.tile([1, 1], F32)
            nc.vector.memset(ident1, 1.0)
            ones_f = persist.tile([128, 1], F32)
            nc.vector.memset(ones_f, 1.0)
            ones = persist.tile([128, 1], F32R)
            nc.vector.tensor_copy(out=ones, in_=ones_f)
            bias = persist.tile([128, D], F32)
            bias_bcast = bass.AP(
                tensor=bout_d, offset=0,
                ap=[[0, 128], [1, D]],
            )
            nc.sync.dma_start(out=bias, in_=bias_bcast)
            # K^T is written straight from phase A matmuls and stays
            # resident for all of phase B (no DRAM roundtrip)
            Ksb = persist.tile([128, DT, N], F32R)

            # ---------------- Phase A ----------------
            with tc.tile_pool(name="pa_big", bufs=1) as pa, \
                 tc.tile_pool(name="pa_stage", bufs=3) as stage, \
                 tc.tile_pool(name="pa_pst", bufs=2, space="PSUM") as pst, \
                 tc.tile_pool(name="pa_psmm", bufs=6, space="PSUM") as psmm:

                WH = pa.tile([128, 4, 3 * D], F32R)   # w rows 256:768
                WL = pa.tile([128, 4, 3 * D], F16)   # w rows 0:256 + 768:1024
                XTH = pa.tile([128, 4, N], F32R)      # x^T high band
                XTL = pa.tile([128, 4, N], F16)      # x^T low bands

                def emit_xpipe(f):
                    # x tiles DMA'd, PE-transposed; low bands cast in the copy
                    for tt in range(4 * f, 4 * f + 4):
                        xn = stage.tile([128, D], F32, tag="xnat")
                        # two half-row DMAs spread across HWDGE queues
                        nc.sync.dma_start(
                            out=xn[:, :512],
                            in_=x_d.ap()[tt * 128:(tt + 1) * 128, :512])
                        nc.sync.dma_start(
                            out=xn[:, 512:],
                            in_=x_d.ap()[tt * 128:(tt + 1) * 128, 512:])
                        tsl = slice(tt * 128, (tt + 1) * 128)
                        for j in range(4):  # high band k-tiles
                            tp = pst.tile([128, 128], F32, tag="tp")
                            nc.tensor.transpose(
                                tp, xn[:, 256 + j * 128: 256 + (j + 1) * 128], ident)
                            nc.any.tensor_copy(out=XTH[:, j, tsl], in_=tp)
                        for li in range(4):  # low band k-tiles (cast in copy)
                            r0 = LOW_ROWS[li]
                            tp = pst.tile([128, 128], F32, tag="tp")
                            nc.tensor.transpose(tp, xn[:, r0:r0 + 128], ident)
                            nc.any.tensor_copy(out=XTL[:, li, tsl], in_=tp)

                # x DMAs + transposes first (independent of W); W loads follow
                emit_xpipe(0)
                for li in range(4):
                    r0 = LOW_ROWS[li]
                    for c in range(6):
                        ws = stage.tile([128, 512], F32, tag="qkvout")
                        nc.gpsimd.dma_start(
                            out=ws,
                            in_=wqkv_d.ap()[r0:r0 + 128, c * 512:(c + 1) * 512])
                        nc.any.tensor_copy(
                            out=WL[:, li, c * 512:(c + 1) * 512], in_=ws)  # ->f16
                for hi in range(4):
                    nc.gpsimd.dma_start(
                        out=WH[:, hi],
                        in_=wqkv_d.ap()[256 + hi * 128:384 + hi * 128]
                        .rearrange("(t p) c -> p t c", p=128).bitcast(F32R))

                # QKV matmuls per 512-token group, x-pipe one group ahead
                for f in range(4):
                    if f + 1 < 4:
                        emit_xpipe(f + 1)
                    tok = slice(f * 512, (f + 1) * 512)
                    # Q^T and K^T: lhsT = w tile (stationary), rhs = x^T
                    for m in range(16):  # 8 Q dh-tiles then 8 K dh-tiles
                        csl = slice(m * 128, (m + 1) * 128)
                        ps = psmm.tile([128, 512], F32, tag="mm")
                        for ki, kt in enumerate(KORDER):
                            low, li = KMAP[kt]
                            if low:
                                nc.tensor.matmul(
                                    ps, WL[:, li, csl], XTL[:, li, tok],
                                    start=(ki == 0), stop=(ki == DT - 1))
                            else:
                                nc.tensor.matmul(
                                    ps, WH[:, li, csl],
                                    XTH[:, li, tok],
                                    start=(ki == 0), stop=(ki == DT - 1))
                        if m < 8:
                            st = stage.tile([128, 512], F32R, tag="qkvout")
                            nc.any.tensor_copy(out=st, in_=ps)
                            nc.sync.dma_start(
                                out=qT[m * 128:(m + 1) * 128, tok], in_=st)
                        else:
                            nc.any.tensor_copy(out=Ksb[:, m - 8, tok], in_=ps)
                    # V natural: lhsT = x^T tile (stationary), rhs = w_v
                    for t in range(4 * f, 4 * f + 4):
                        tsl = slice(t * 128, (t + 1) * 128)
                        for h in range(2):
                            vsl = slice(2 * D + h * 512, 2 * D + (h + 1) * 512)
                            ps = psmm.tile([128, 512], F32, tag="mm")
                            for ki, kt in enumerate(KORDER):
                                low, li = KMAP[kt]
                                if low:
                                    nc.tensor.matmul(
                                        ps, XTL[:, li, tsl], WL[:, li, vsl],
                                        start=(ki == 0), stop=(ki == DT - 1))
                                else:
                                    nc.tensor.matmul(
                                        ps, XTH[:, li, tsl],
                                        WH[:, li, vsl],
                                        start=(ki == 0), stop=(ki == DT - 1))
                            st = stage.tile([128, 512], F32R, tag="qkvout")
                            nc.any.tensor_copy(out=st, in_=ps)
                            nc.sync.dma_start(
                                out=vN[t * 128:(t + 1) * 128, h * 512:(h + 1) * 512],
                                in_=st)

            # ---------------- Phase B ----------------
            with tc.tile_pool(name="pb_big", bufs=1) as pb, \
                 tc.tile_pool(name="pb_q", bufs=2) as pq, \
                 tc.tile_pool(name="pb_p", bufs=3) as ppt, \
                 tc.tile_pool(name="pb_y", bufs=1) as py, \
                 tc.tile_pool(name="pb_o", bufs=3) as po, \
                 tc.tile_pool(name="pb_misc", bufs=2) as pmisc, \
                 tc.tile_pool(name="pb_psy", bufs=1, space="PSUM") as psy, \
                 tc.tile_pool(name="pb_pssum", bufs=1, space="PSUM") as pssum, \
                 tc.tile_pool(name="pb_pss", bufs=3, space="PSUM") as pss:

                # V load chunked so block 0's Y matmuls start early; K is
                # already resident. W_out^T is built here on the PE (idle
                # during the V load) and stays resident for all blocks.
                qb_pre = pq.tile([128, DT, QBLK], F32R, tag="qb")
                nc.sync.dma_start(
                    out=qb_pre,
                    in_=qT[:, 0:QBLK].rearrange("(t p) q -> p t q", p=128))
                Vsb = pb.tile([128, NT, D], F32R)
                for c in range(8):
                    nc.sync.dma_start(
                        out=Vsb[:, 2 * c:2 * c + 2],
                        in_=vN[c * 256:(c + 1) * 256].rearrange(
                            "(t p) d -> p t d", p=128))
                woutT = pb.tile([128, DT, D], F32R)

                def emit_woutT():
                    for et in range(8):
                        wn = pq.tile([128, D], F32, tag="woutnat")
                        nc.sync.dma_start(
                            out=wn, in_=wout_d.ap()[et * 128:(et + 1) * 128])
                        for j in range(8):
                            tp = pss.tile([128, QBLK], F32, tag="small")
                            nc.tensor.transpose(
                                tp[:, :128], wn[:, j * 128:(j + 1) * 128], ident)
                            nc.any.tensor_copy(
                                out=woutT[:, j, et * 128:(et + 1) * 128],
                                in_=tp[:, :128])

                for b in range(NBLK):
                    q0 = b * QBLK
                    if b == 0:
                        Qb = qb_pre
                    else:
                        Qb = pq.tile([128, DT, QBLK], F32R, tag="qb")
                        nc.sync.dma_start(
                            out=Qb,
                            in_=qT[:, q0:q0 + QBLK].rearrange(
                                "(t p) q -> p t q", p=128))

                    yt_ps = psy.tile([128, DT, QBLK], F32, tag="yt")
                    # zero the accumulator and use start=False on every matmul:
                    # correct for any PE ordering / stale has_written state
                    # (accumulate-onto-0 and overwrite-with-product agree)
                    nc.vector.memset(yt_ps, 0.0)
                    sums_ps = pssum.tile([1, QBLK], F32, tag="sums")

                    def emit_s(j):
                        ksl = slice(j * 128, (j + 1) * 128)
                        s_ps = pss.tile([128, QBLK], F32, tag="small")
                        for kt in range(DT):
                            nc.tensor.matmul(
                                s_ps, Ksb[:, kt, ksl],
                                Qb[:, kt],
                                start=(kt == 0), stop=(kt == DT - 1))
                        pt = ppt.tile([128, QBLK], F32R, tag="pt")
                        nc.scalar.activation(
                            out=pt, in_=s_ps,
                            func=mybir.ActivationFunctionType.Exp,
                            scale=1.0 / 32.0)
                        return pt

                    def emit_y(j, pt):
                        for m in range(DT):
                            nc.tensor.matmul(
                                yt_ps[:, m],
                                Vsb[:, j, m * 128:(m + 1) * 128],
                                pt,
                                start=False, stop=(j == NT - 1),
                                skip_group_check=True)
                        nc.tensor.matmul(
                            sums_ps, ones, pt,
                            start=(j == 0), stop=(j == NT - 1),
                            skip_group_check=True)

                    # software pipeline: PE computes S(j+1) while ACT exps j
                    pt_prev = emit_s(0)
                    for j in range(1, NT):
                        pt_j = emit_s(j)
                        emit_y(j - 1, pt_prev)
                        pt_prev = pt_j
                    emit_y(NT - 1, pt_prev)

                    if b == 0:
                        # W_out^T built here: PE was busy on S/Y above while
                        # w_out streamed in; needed first by phase 4 below
                        emit_woutT()

                    # 1/rowsum as a per-partition column, via PE transpose
                    sums_sb = pmisc.tile([1, QBLK], F32, tag="sums_sb")
                    nc.any.tensor_copy(out=sums_sb, in_=sums_ps)
                    recip = pmisc.tile([128, 2], F32, tag="recip")
                    for t in range(2):
                        rp = pss.tile([128, 1], F32, tag="small")
                        nc.tensor.transpose(
                            rp, sums_sb[0:1, t * 128:(t + 1) * 128], ident1)
                        nc.vector.reciprocal(out=recip[:, t:t + 1], in_=rp)

                    yt_sb = py.tile([128, DT, QBLK], F32R, tag="yt_sb")
                    for m in range(DT):
                        nc.any.tensor_copy(out=yt_sb[:, m], in_=yt_ps[:, m])

                    # out projection + fused epilogue
                    for e4 in range(4):
                        esl = slice(e4 * 256, (e4 + 1) * 256)
                        for t in range(2):
                            qsl = slice(t * 128, (t + 1) * 128)
                            o_ps = pss.tile([128, QBLK], F32, tag="small")
                            for kt in range(DT):
                                nc.tensor.matmul(
                                    o_ps, yt_sb[:, kt, qsl],
                                    woutT[:, kt, esl],
                                    start=(kt == 0), stop=(kt == DT - 1))
                            o_sb = po.tile([128, 256], F32, tag="osb")
                            nc.vector.scalar_tensor_tensor(
                                out=o_sb, in0=o_ps, scalar=recip[:, t:t + 1],
                                in1=bias[:, esl],
                                op0=mybir.AluOpType.mult,
                                op1=mybir.AluOpType.add)
                            nc.sync.dma_start(
                                out=out_d.ap()[q0 + t * 128: q0 + (t + 1) * 128, esl],
                                in_=o_sb)
    nc.finalize()
    return nc


_NC = None


def kernel(**inputs) -> np.ndarray:
    global _NC
    if _NC is None:
        _NC = build_nc()
    x = np.ascontiguousarray(inputs["x"], dtype=np.float32)
    w = np.ascontiguousarray(inputs["weight_qkv"], dtype=np.float32)
    ow = np.ascontiguousarray(inputs["out_w"], dtype=np.float32)
    ob = np.ascontiguousarray(inputs["out_b"], dtype=np.float32)
    in_maps = [
        {"x": x[i], "weight_qkv": w, "out_w": ow, "out_b": ob} for i in range(B)
    ]
    res = run_bass_kernel_spmd(_NC, in_maps, core_ids=list(range(B)))
    return np.stack([res.results[i]["out"] for i in range(B)], axis=0)


if __name__ == "__main__":
    rng = np.random.default_rng(0)
    ins = {
        "x": rng.standard_normal((B, N, D), dtype=np.float32),
        "weight_qkv": (rng.standard_normal((D, 3 * D)) * D ** -0.5).astype(np.float32),
        "out_w": (rng.standard_normal((D, D)) * D ** -0.5).astype(np.float32),
        "out_b": (rng.standard_normal(D) * 0.01).astype(np.float32),
    }
    out = kernel(**ins)
    print(out.shape, out.dtype)


# revision 31
# speedup vs baseline: 12247.4613x; 1.0025x over previous
"""Trainium2 Bass kernel for nn_MultiHeadAttention_40286793236532.

Single-head attention with a mixed-precision QKV projection:
  qkv = x @ w_qkv   (contraction split fp16 | fp32 | fp16 over bands
                     [0:256) [256:768) [768:1024))
  q, k, v = split(qkv); s = softmax(q k^T / 32); out = (s v) @ w_out^T + b

Sharding: data-parallel over batch B=8 -> one batch element per NeuronCore,
no collectives. Each core runs the identical program on its x-slice.

Per-core algorithm (N=2048 tokens, d=1024):
  Phase A: transpose x on-chip (PE transpose; low bands cast to fp16 in the
    PSUM->SBUF copy), then QKV matmuls: fp16 matmuls for the low contraction
    bands, float32r (full PE rate, ~1.6e-4 rel err) for the high band; fp16
    k-tiles run first in each accumulation chain so compute starts before the
    fp32 weights finish streaming. K^T is written straight into a resident
    SBUF tile (no DRAM roundtrip); Q^T [d, N] and V [N, d] spill to DRAM
    scratch (SBUF cannot hold Q, K, V and the x/w working set at once).
  Phase B: K^T resident, V loaded in chunks (so block 0 starts immediately),
    W_out^T built on-chip by PE transposes inside block 0 (PE is busy on S/Y
    while w_out streams in) and kept resident. Per 256-query block:
    S^T tiles = K-tile^T . Q-block (f32r, keys on partitions), exp on ACT with
    scale=1/32 folded in (no max subtraction: |logits| <~ 7, safe in fp32),
    software-pipelined so PE computes S(j+1) while ACT exps S(j). Y^T
    accumulates over key tiles in PSUM (lhsT = V tile, memset + start=False
    so any PE ordering is correct); row sums via a ones-column matmul, then
    out = Y^T.T @ W_out^T with a fused epilogue (x * 1/rowsum + bias).

Timing feedback came from the cost-model timeline simulator (NTFF profiling
is unavailable under this axon client): 758 -> 588 us/core predicted, PE
occupancy 83%, vs ~460 us full-rate matmul roofline.
"""

import numpy as np

import concourse.bacc as bacc
import concourse.bass as bass
import concourse.mybir as mybir
import concourse.tile as tile
from concourse.bass_utils import run_bass_kernel_spmd
from concourse.masks import make_identity

F32 = mybir.dt.float32
F32R = mybir.dt.float32r
F16 = mybir.dt.float16

B, N, D = 8, 2048, 1024
KL = 256          # low-precision band width (each side)
NT = N // 128     # 16 token tiles
DT = D // 128     # 8 contraction k-tiles
QBLK = 256        # queries per phase-B block
NBLK = N // QBLK  # 8 blocks
# global k-tile index -> (is_low, local index within WH/WL, XTH/XTL)
# low k-tiles: 0,1 (cols 0:256) and 6,7 (cols 768:1024); high: 2..5
KMAP = {0: (True, 0), 1: (True, 1), 2: (False, 0), 3: (False, 1),
        4: (False, 2), 5: (False, 3), 6: (True, 2), 7: (True, 3)}
KORDER = [0, 1, 6, 7, 2, 3, 4, 5]  # fp16 tiles first: mms start before WH lands
LOW_ROWS = [0, 128, 768, 896]  # first row of each low k-tile in w_qkv/x cols


def _r(dt_, ap):
    """bitcast an fp32 AP to float32r for full-rate PE matmuls"""
    return ap.bitcast(dt_)


def build_nc():
    nc = bacc.Bacc()
    x_d = nc.dram_tensor("x", [N, D], F32, kind="ExternalInput")
    wqkv_d = nc.dram_tensor("weight_qkv", [D, 3 * D], F32, kind="ExternalInput")
    wout_d = nc.dram_tensor("out_w", [D, D], F32, kind="ExternalInput")
    bout_d = nc.dram_tensor("out_b", [D], F32, kind="ExternalInput")
    out_d = nc.dram_tensor("out", [N, D], F32, kind="ExternalOutput")

    with tile.TileContext(nc) as tc:
        with tc.tile_pool(name="dram", bufs=1, space="DRAM") as dram, \
             tc.tile_pool(name="persist", bufs=1) as persist:
            qT = dram.tile([D, N], F32R)      # Q^T scratch
            vN = dram.tile([N, D], F32R)      # V natural scratch

            ident = persist.tile([128, 128], F32)
            make_identity(nc, ident)
            ident1 = persist.tile([1, 1], F32)
            nc.vector.memset(ident1, 1.0)
            ones_f = persist.tile([128, 1], F32)
            nc.vector.memset(ones_f, 1.0)
            ones = persist.tile([128, 1], F32R)
            nc.vector.tensor_copy(out=ones, in_=ones_f)
            bias = persist.tile([128, D], F32)
            bias_bcast = bass.AP(
                tensor=bout_d, offset=0,
                ap=[[0, 128], [1, D]],
            )
            nc.sync.dma_start(out=bias, in_=bias_bcast)
            # K^T is written straight from phase A matmuls and stays
            # resident for all of phase B (no DRAM roundtrip)
            Ksb = persist.tile([128, DT, N], F32R)

            # ---------------- Phase A ----------------
            with tc.tile_pool(name="pa_big", bufs=1) as pa, \
                 tc.tile_pool(name="pa_stage", bufs=3) as stage, \
                 tc.tile_pool(name="pa_pst", bufs=2, space="PSUM") as pst, \
                 tc.tile_pool(name="pa_psmm", bufs=6, space="PSUM") as psmm:

                WH = pa.tile([128, 4, 3 * D], F32R)   # w rows 256:768
                WL = pa.tile([128, 4, 3 * D], F16)   # w rows 0:256 + 768:1024
                XTH = pa.tile([128, 4, N], F32R)      # x^T high band
                XTL = pa.tile([128, 4, N], F16)      # x^T low bands

                def emit_xpipe(f):
                    # x tiles DMA'd, PE-transposed; low bands cast in the copy
                    for tt in range(4 * f, 4 * f + 4):
                        xn = stage.tile([128, D], F32, tag="xnat")
                        # two half-row DMAs spread across HWDGE queues
                        nc.sync.dma_start(
                            out=xn[:, :512],
                            in_=x_d.ap()[tt * 128:(tt + 1) * 128, :512])
                        nc.sync.dma_start(
                            out=xn[:, 512:],
                            in_=x_d.ap()[tt * 128:(tt + 1) * 128, 512:])
                        tsl = slice(tt * 128, (tt + 1) * 128)
                        for j in range(4):  # high band k-tiles
                            tp = pst.tile([128, 128], F32, tag="tp")
                            nc.tensor.transpose(
                                tp, xn[:, 256 + j * 128: 256 + (j + 1) * 128], ident)
                            nc.scalar.copy(out=XTH[:, j, tsl], in_=tp)
                        for li in range(4):  # low band k-tiles (cast in copy)
                            r0 = LOW_ROWS[li]
                            tp = pst.tile([128, 128], F32, tag="tp")
                            nc.tensor.transpose(tp, xn[:, r0:r0 + 128], ident)
                            nc.scalar.copy(out=XTL[:, li, tsl], in_=tp)

                # x DMAs + transposes first (independent of W); W loads follow
                emit_xpipe(0)
                for li in range(4):
                    r0 = LOW_ROWS[li]
                    for c in range(6):
                        ws = stage.tile([128, 512], F32, tag="qkvout")
                        nc.gpsimd.dma_start(
                            out=ws,
                            in_=wqkv_d.ap()[r0:r0 + 128, c * 512:(c + 1) * 512])
                        nc.vector.tensor_copy(
                            out=WL[:, li, c * 512:(c + 1) * 512], in_=ws)  # ->f16
                for hi in range(4):
                    nc.gpsimd.dma_start(
                        out=WH[:, hi],
                        in_=wqkv_d.ap()[256 + hi * 128:384 + hi * 128]
                        .rearrange("(t p) c -> p t c", p=128).bitcast(F32R))

                # QKV matmuls per 512-token group, x-pipe one group ahead
                for f in range(4):
                    if f + 1 < 4:
                        emit_xpipe(f + 1)
                    tok = slice(f * 512, (f + 1) * 512)
                    # Q^T and K^T: lhsT = w tile (stationary), rhs = x^T
                    for m in range(16):  # 8 Q dh-tiles then 8 K dh-tiles
                        csl = slice(m * 128, (m + 1) * 128)
                        ps = psmm.tile([128, 512], F32, tag="mm")
                        for ki, kt in enumerate(KORDER):
                            low, li = KMAP[kt]
                            if low:
                                nc.tensor.matmul(
                                    ps, WL[:, li, csl], XTL[:, li, tok],
                                    start=(ki == 0), stop=(ki == DT - 1))
                            else:
                                nc.tensor.matmul(
                                    ps, WH[:, li, csl],
                                    XTH[:, li, tok],
                                    start=(ki == 0), stop=(ki == DT - 1))
                        if m < 8:
                            st = stage.tile([128, 512], F32R, tag="qkvout")
                            nc.vector.tensor_copy(out=st, in_=ps)
                            nc.sync.dma_start(
                                out=qT[m * 128:(m + 1) * 128, tok], in_=st)
                        else:
                            nc.vector.tensor_copy(out=Ksb[:, m - 8, tok], in_=ps)
                    # V natural: lhsT = x^T tile (stationary), rhs = w_v
                    for t in range(4 * f, 4 * f + 4):
                        tsl = slice(t * 128, (t + 1) * 128)
                        for h in range(2):
                            vsl = slice(2 * D + h * 512, 2 * D + (h + 1) * 512)
                            ps = psmm.tile([128, 512], F32, tag="mm")
                            for ki, kt in enumerate(KORDER):
                                low, li = KMAP[kt]
                                if low:
                                    nc.tensor.matmul(
                                        ps, XTL[:, li, tsl], WL[:, li, vsl],
                                        start=(ki == 0), stop=(ki == DT - 1))
                                else:
                                    nc.tensor.matmul(
                                        ps, XTH[:, li, tsl],
                                        WH[:, li, vsl],
                                        start=(ki == 0), stop=(ki == DT - 1))
                            st = stage.tile([128, 512], F32R, tag="qkvout")
                            nc.any.tensor_copy(out=st, in_=ps)
                            nc.sync.dma_start(
                                out=vN[t * 128:(t + 1) * 128, h * 512:(h + 1) * 512],
                                in_=st)

            # ---------------- Phase B ----------------
            with tc.tile_pool(name="pb_big", bufs=1) as pb, \
                 tc.tile_pool(name="pb_q", bufs=2) as pq, \
                 tc.tile_pool(name="pb_p", bufs=3) as ppt, \
                 tc.tile_pool(name="pb_y", bufs=1) as py, \
                 tc.tile_pool(name="pb_o", bufs=3) as po, \
                 tc.tile_pool(name="pb_misc", bufs=2) as pmisc, \
                 tc.tile_pool(name="pb_psy", bufs=1, space="PSUM") as psy, \
                 tc.tile_pool(name="pb_pssum", bufs=1, space="PSUM") as pssum, \
                 tc.tile_pool(name="pb_pss", bufs=3, space="PSUM") as pss:

                # V load chunked so block 0's Y matmuls start early; K is
                # already resident. W_out^T is built here on the PE (idle
                # during the V load) and stays resident for all blocks.
                qb_pre = pq.tile([128, DT, QBLK], F32R, tag="qb")
                nc.sync.dma_start(
                    out=qb_pre,
                    in_=qT[:, 0:QBLK].rearrange("(t p) q -> p t q", p=128))
                Vsb = pb.tile([128, NT, D], F32R)
                for c in range(8):
                    nc.sync.dma_start(
                        out=Vsb[:, 2 * c:2 * c + 2],
                        in_=vN[c * 256:(c + 1) * 256].rearrange(
                            "(t p) d -> p t d", p=128))
                woutT = pb.tile([128, DT, D], F32R)

                def emit_woutT():
                    for et in range(8):
                        wn = pq.tile([128, D], F32, tag="woutnat")
                        nc.sync.dma_start(
                            out=wn, in_=wout_d.ap()[et * 128:(et + 1) * 128])
                        for j in range(8):
                            tp = pss.tile([128, QBLK], F32, tag="small")
                            nc.tensor.transpose(
                                tp[:, :128], wn[:, j * 128:(j + 1) * 128], ident)
                            nc.vector.tensor_copy(
                                out=woutT[:, j, et * 128:(et + 1) * 128],
                                in_=tp[:, :128])

                def emit_phase4(q0, yt_sb, recip):
                    for e4 in range(4):
                        esl = slice(e4 * 256, (e4 + 1) * 256)
                        for t in range(2):
                            qsl = slice(t * 128, (t + 1) * 128)
                            o_ps = pss.tile([128, QBLK], F32, tag="small")
                            for kt in range(DT):
                                nc.tensor.matmul(
                                    o_ps, yt_sb[:, kt, qsl],
                                    woutT[:, kt, esl],
                                    start=(kt == 0), stop=(kt == DT - 1))
                            o_sb = po.tile([128, 256], F32, tag="osb")
                            nc.vector.scalar_tensor_tensor(
                                out=o_sb, in0=o_ps, scalar=recip[:, t:t + 1],
                                in1=bias[:, esl],
                                op0=mybir.AluOpType.mult,
                                op1=mybir.AluOpType.add)
                            nc.sync.dma_start(
                                out=out_d.ap()[q0 + t * 128: q0 + (t + 1) * 128, esl],
                                in_=o_sb)

                pending = None
                for b in range(NBLK):
                    q0 = b * QBLK
                    if b == 0:
                        Qb = qb_pre
                    else:
                        Qb = pq.tile([128, DT, QBLK], F32R, tag="qb")
                        nc.sync.dma_start(
                            out=Qb,
                            in_=qT[:, q0:q0 + QBLK].rearrange(
                                "(t p) q -> p t q", p=128))

                    yt_ps = psy.tile([128, DT, QBLK], F32, tag="yt")
                    # zero the accumulator and use start=False on every matmul:
                    # correct for any PE ordering / stale has_written state
                    # (accumulate-onto-0 and overwrite-with-product agree)
                    nc.vector.memset(yt_ps, 0.0)
                    sums_ps = pssum.tile([1, QBLK], F32, tag="sums")

                    def emit_s(j):
                        ksl = slice(j * 128, (j + 1) * 128)
                        s_ps = pss.tile([128, QBLK], F32, tag="small")
                        for kt in range(DT):
                            nc.tensor.matmul(
                                s_ps, Ksb[:, kt, ksl],
                                Qb[:, kt],
                                start=(kt == 0), stop=(kt == DT - 1))
                        pt = ppt.tile([128, QBLK], F32R, tag="pt")
                        nc.scalar.activation(
                            out=pt, in_=s_ps,
                            func=mybir.ActivationFunctionType.Exp,
                            scale=1.0 / 32.0)
                        return pt

                    def emit_y(j, pt):
                        for m in range(DT):
                            nc.tensor.matmul(
                                yt_ps[:, m],
                                Vsb[:, j, m * 128:(m + 1) * 128],
                                pt,
                                start=False, stop=(j == NT - 1),
                                skip_group_check=True)
                        nc.tensor.matmul(
                            sums_ps, ones, pt,
                            start=(j == 0), stop=(j == NT - 1),
                            skip_group_check=True)

                    # software pipeline: PE computes S(j+1) while ACT exps j
                    pt_prev = emit_s(0)
                    for j in range(1, NT):
                        pt_j = emit_s(j)
                        emit_y(j - 1, pt_prev)
                        pt_prev = pt_j
                    emit_y(NT - 1, pt_prev)

                    if b == 0:
                        # W_out^T built here: PE was busy on S/Y above while
                        # w_out streamed in; needed first by phase 4 below
                        emit_woutT()

                    # 1/rowsum as a per-partition column, via PE transpose
                    sums_sb = pmisc.tile([1, QBLK], F32, tag="sums_sb")
                    nc.vector.tensor_copy(out=sums_sb, in_=sums_ps)
                    recip = pmisc.tile([128, 2], F32, tag="recip")
                    for t in range(2):
                        rp = pss.tile([128, 1], F32, tag="small")
                        nc.tensor.transpose(
                            rp, sums_sb[0:1, t * 128:(t + 1) * 128], ident1)
                        nc.vector.reciprocal(out=recip[:, t:t + 1], in_=rp)

                    yt_sb = py.tile([128, DT, QBLK], F32R, tag="yt_sb")
                    for m in range(DT):
                        nc.vector.tensor_copy(out=yt_sb[:, m], in_=yt_ps[:, m])

                    # out projection + fused epilogue
                    emit_phase4(q0, yt_sb, recip)
    nc.finalize()
    return nc


_NC = None


def kernel(**inputs) -> np.ndarray:
    global _NC
    if _NC is None:
        _NC = build_nc()
    x = np.ascontiguousarray(inputs["x"], dtype=np.float32)
    w = np.ascontiguousarray(inputs["weight_qkv"], dtype=np.float32)
    ow = np.ascontiguousarray(inputs["out_w"], dtype=np.float32)
    ob = np.ascontiguousarray(inputs["out_b"], dtype=np.float32)
    in_maps = [
        {"x": x[i], "weight_qkv": w, "out_w": ow, "out_b": ob} for i in range(B)
    ]
    res = run_bass_kernel_spmd(_NC, in_maps, core_ids=list(range(B)))
    return np.stack([res.results[i]["out"] for i in range(B)], axis=0)


if __name__ == "__main__":
    rng = np.random.default_rng(0)
    ins = {
        "x": rng.standard_normal((B, N, D), dtype=np.float32),
        "weight_qkv": (rng.standard_normal((D, 3 * D)) * D ** -0.5).astype(np.float32),
        "out_w": (rng.standard_normal((D, D)) * D ** -0.5).astype(np.float32),
        "out_b": (rng.standard_normal(D) * 0.01).astype(np.float32),
    }
    out = kernel(**ins)
    print(out.shape, out.dtype)
